# revision 34
# baseline (speedup 1.0000x reference)
"""Distributed Trainium2 (Bass/Tile) kernel for a pre-LN transformer block.

Reference computation (per batch element):
    xn = LN1(x); q,k,v = per-head projections of xn
    attn = causal-softmax(q k^T / sqrt(dh)) v
    x1 = x + concat_heads(attn) @ w_proj + b_proj
    out = x1 + relu(LN2(x1) @ w1 + b1) @ w2 + b2

Sharding over 8 NeuronCores: core c handles batch b=c//4 and head group
g=c%4 (4 of 16 heads).  Attention is head-parallel; a per-window
AllGather over each 4-core group collects all 16 heads of one 512-row
window on its owner rank, which then runs projection + FFN
sequence-parallel (512 rows per core).  The host assembles the 8
[512,1024] outputs into the full [2,2048,1024] result.

Schedule notes:
- LayerNorm-1 is merged into the attention window pipeline: tiles are
  normalized and DMA-transposed just ahead of the QKV projections that
  consume them, so the tensor engine starts ~10us into the kernel.
- Score matmuls are emitted as head PAIRS on alternating 64-partition
  halves of the PE array (row tiling), so each adjacent pair of
  64-contraction matmuls runs concurrently.
- The softmax tiles (exp) and V are fp8e4; exp carries a -ln32 bias so
  outlier scores cannot overflow fp8's +-240 (numerator and denominator
  scale together).  The causal mask is a DVE multiply with precomputed
  0/1 tiles (gpsimd stays free for collective service).
- Collective discipline (the core of this kernel's schedule): the
  runtime's first-CC barrier costs up to ~130us cold, and the scheduler
  plants per-engine "Collectives>=k" observation waits at the first DGE
  guard past each collective's MODELED completion.  Every
  collective_compute is therefore pinned late in the modeled timeline
  (tile_wait_until), all CC staging/stitch DMAs live on gpsimd's
  software DGE or the scalar hwdge (never SP), and the stitch + blend +
  projection tail is pinned after the attention pipeline.  The SP
  stream carries zero CC-dependent work, so a slow gather can never
  stall compute.
- A dummy AllGather issued at the very top starts the barrier a few us
  in; per-window gathers (one per 512-row window, 4 heads each) follow
  as attention produces them, and only the last window's gather is
  exposed, bridged by a PE warm chain.  Warm filler matmuls trickle
  through attention so the HAM clock gate stays at full rate.
"""

import numpy as np
import ml_dtypes

import concourse.bass as bass
import concourse.mybir as mybir
import concourse.tile as tile
from contextlib import ExitStack
from concourse import bacc
from concourse.bass_utils import run_bass_kernel_spmd

T = 2048          # sequence length
D = 1024          # embedding dim
H = 16            # total heads
DH = 64           # head dim
HL = 4            # heads per core
TG = 512          # rows per core in the FFN phase
DF = 4096         # FFN hidden dim
EPS = 1e-5
N_CORES = 8
N_WARM0 = 96      # kernel-start PE warm burst
N_BRIDGE = 80     # PE warm chain over the final AllGather

f32 = mybir.dt.float32
bf16 = mybir.dt.bfloat16
fp8 = mybir.dt.float8e4
AF = mybir.ActivationFunctionType
ALU = mybir.AluOpType
BF16 = ml_dtypes.bfloat16
F8 = ml_dtypes.float8_e4m3


def _f8(x):
    # TRN fp8e4 saturates at +-240 (not OCP's 448); clip before the cast
    return np.clip(x, -240.0, 240.0).astype(F8)


def _rsqrtN(nc, pool, ve, magic_ap, n):
    """DVE-only rsqrt over a [128, n] tile of (var+eps) values: quake-style
    bit seed + 2 Newton steps.  Keeps sqrt off the scalar engine so one
    activation-table set (exp/identity/relu) serves the whole kernel."""
    i32 = mybir.dt.int32
    ti = pool.tile([128, n], i32, tag="rsq_ti")
    nc.vector.tensor_scalar(ti[:], ve[:].bitcast(i32), 1, None,
                            ALU.logical_shift_right)
    y0i = pool.tile([128, n], i32, tag="rsq_y0")
    nc.vector.tensor_tensor(y0i[:], magic_ap, ti[:], ALU.subtract)
    rstd = pool.tile([128, n], f32, tag="rsq_out")
    tmp = pool.tile([128, n], f32, tag="rsq_tmp")
    y = y0i[:].bitcast(f32)
    for it in range(2):
        nc.vector.tensor_tensor(tmp[:], y, y, ALU.mult)
        nc.vector.tensor_tensor(tmp[:], tmp[:], ve[:], ALU.mult)
        nc.vector.tensor_scalar(tmp[:], tmp[:], -0.5, 1.5, ALU.mult, ALU.add)
        nc.vector.tensor_tensor(rstd[:], y, tmp[:], ALU.mult)
        y = rstd[:]
    return rstd


def _ln_stats(nc, pool, xt, width, magic_ap):
    """Per-partition mean/var over `width` free elements -> (rstd, neg_mu_rstd)."""
    nchunk = width // 512
    bns = pool.tile([128, nchunk, 6], f32, tag="bns")
    for i in range(nchunk):
        nc.vector.bn_stats(bns[:, i, :], xt[:, i * 512:(i + 1) * 512])
    agg = pool.tile([128, 2], f32, tag="agg")
    nc.vector.bn_aggr(agg[:], bns[:].rearrange("p a b -> p (a b)"))
    ve = pool.tile([128, 1], f32, tag="ve")
    nc.vector.tensor_scalar(ve[:], agg[:, 1:2], EPS, None, ALU.add)
    rstd = _rsqrtN(nc, pool, ve, magic_ap[:, 0:1], 1)
    nmr = pool.tile([128, 1], f32, tag="nmr")
    nc.vector.tensor_scalar(nmr[:], agg[:, 0:1], rstd[:], -1.0, ALU.mult, ALU.mult)
    return rstd, nmr


def _build():
    nc = bacc.Bacc("TRN2", target_bir_lowering=False, debug=False,
                   num_devices=N_CORES)

    x_in = nc.dram_tensor("x", [T, D], bf16, kind="ExternalInput")
    wq_in = nc.dram_tensor("wq", [128, 8, HL * DH], bf16, kind="ExternalInput")
    wk_in = nc.dram_tensor("wk", [128, 8, HL * DH], bf16, kind="ExternalInput")
    wv_in = nc.dram_tensor("wv", [128, 8, HL * DH], bf16, kind="ExternalInput")
    qb_in = nc.dram_tensor("qb", [128, 2], f32, kind="ExternalInput")
    kb_in = nc.dram_tensor("kb", [128, 2], f32, kind="ExternalInput")
    vb_in = nc.dram_tensor("vb", [1, HL * DH], f32, kind="ExternalInput")
    wp_in = nc.dram_tensor("wp", [128, 8, D], bf16, kind="ExternalInput")
    w1_in = nc.dram_tensor("w1", [128, 32, 8, 128], bf16, kind="ExternalInput")
    b1_in = nc.dram_tensor("b1", [128, 32], f32, kind="ExternalInput")
    w2_in = nc.dram_tensor("w2", [DF, D], bf16, kind="ExternalInput")
    b2_in = nc.dram_tensor("b2", [1, D], f32, kind="ExternalInput")
    xg_in = nc.dram_tensor("xg", [TG, D], bf16, kind="ExternalInput")
    ws_in = nc.dram_tensor("wsel", [128, 4], f32, kind="ExternalInput")
    out_dram = nc.dram_tensor("out", [TG, D], f32, kind="ExternalOutput")

    GROUPS = [[0, 1, 2, 3], [4, 5, 6, 7]]

    with tile.TileContext(nc) as tc, ExitStack() as top:
        persist = top.enter_context(tc.tile_pool(name="persist", bufs=1))
        mid = top.enter_context(tc.tile_pool(name="mid", bufs=4))
        consts = top.enter_context(tc.tile_pool(name="consts", bufs=1))
        dram = top.enter_context(tc.tile_pool(name="dram", bufs=1, space="DRAM"))
        wpool = top.enter_context(tc.tile_pool(name="p7w", bufs=6))

        # ---- dummy collective at t~0: the runtime's first-CC barrier
        # (all-core sync + CC init, up to ~130us cold) starts the moment the
        # first trigger lands, so fire it before anything else.  Everything
        # CC-dependent is kept off the pipeline engines (see oa_stitch), so
        # this never stalls compute. ----
        dmy_in = dram.tile([1, 128], fp8, name="dmy_in", tag="dmy_in")
        dmy_out = dram.tile([4, 1, 128], fp8, name="dmy_out", tag="dmy_out")
        dmy_sb = consts.tile([1, 128], fp8, tag="dmy_sb")
        nc.gpsimd.memset(dmy_sb[:], 0.0)
        nc.gpsimd.dma_start(dmy_in[:], dmy_sb[:])
        # Dummy collective: absorbs the runtime's first-CC barrier
        # (all-core rendezvous + CC init, ~130us cold).  Pinned late in the
        # *modeled* timeline: the lowering plants a per-engine
        # "Collectives>=k" observation wait at the first DGE guard past
        # each collective's modeled completion, and with a modeled
        # completion of ~130us those land harmlessly among the FFN weight
        # loads instead of in front of the attention pipeline's LN
        # transposes.  gpsimd still reaches this position almost
        # immediately in real time (nothing before it can block), so the
        # real trigger fires a few us in and the barrier starts early.
        with tc.tile_wait_until(0.115):
            nc.gpsimd.collective_compute(
                "AllGather", ALU.bypass, replica_groups=GROUPS,
                ins=[dmy_in[:].opt()], outs=[dmy_out[:].opt()])

        # ---- constants ----
        magict = consts.tile([128, 4], mybir.dt.int32, tag="magict")
        nc.vector.memset(magict[:], 0x5F3759DF)
        b1s = consts.tile([128, 32], f32, tag="b1s")
        nc.sync.dma_start(b1s[:], b1_in[:])
        qbt = consts.tile([128, 2], f32, tag="qbt")
        nc.sync.dma_start(qbt[:], qb_in[:])
        kbt = consts.tile([128, 2], f32, tag="kbt")
        nc.sync.dma_start(kbt[:], kb_in[:])
        vbr = consts.tile([1, HL * DH], f32, tag="vbr")
        nc.sync.dma_start(vbr[:], vb_in[:])
        vbb = consts.tile([128, HL, DH], f32, tag="vbb")
        nc.gpsimd.partition_broadcast(vbb[:].rearrange("p h d -> p (h d)"), vbr[:])
        b2_row = consts.tile([1, D], f32, tag="brow2")
        nc.sync.dma_start(b2_row[:], b2_in[:])
        b2b = consts.tile([128, D], f32, tag="b2b")
        nc.gpsimd.partition_broadcast(b2b[:], b2_row[:])
        wselt = consts.tile([128, 4], f32, tag="wselt")
        nc.sync.dma_start(wselt[:], ws_in[:])
        wp_sb = consts.tile([128, 8, D], bf16, tag="wp_sb")
        wq_sb = consts.tile([128, 8, HL * DH], bf16, tag="wq_sb")
        nc.sync.dma_start(wq_sb[:], wq_in[:])
        wk_sb = consts.tile([128, 8, HL * DH], bf16, tag="wk_sb")
        nc.sync.dma_start(wk_sb[:], wk_in[:])
        wv_sb = consts.tile([128, 8, HL * DH], bf16, tag="wv_sb")
        nc.sync.dma_start(wv_sb[:], wv_in[:])
        # causal 0/1 masks for the 4 diagonal key-tiles of a 512-query
        # window (d = key_tile - 4*window): built once on gpsimd, applied
        # in-loop on DVE (keeps gpsimd free of mid-attention work so CC
        # waits can never stall the softmax pipeline)
        cmask = consts.tile([128, 4, 512], fp8, tag="cmask")
        nc.vector.memset(cmask[:], 1.0)
        for dd in range(4):
            nc.gpsimd.affine_select(
                out=cmask[:, dd, :], in_=cmask[:, dd, :],
                compare_op=ALU.is_ge, fill=0.0,
                base=-128 * dd, pattern=[[1, 512]],
                channel_multiplier=-1)
        ones64 = consts.tile([1, DH], bf16, tag="ones64")
        nc.vector.memset(ones64[:], 1.0)
        expb = consts.tile([128, 1], f32, tag="expb")
        nc.vector.memset(expb[:], -3.4657359027997265)

        # ---- kernel-start PE warm burst: trips the HAM clock gate while
        # the first LN tiles (no PE work) stream in ----
        with tc.tile_pool(name="warm0", bufs=1, space="PSUM") as wps:
            pw = wps.tile([128, 256], f32, tag="w0", name="w0_acc")
            for dd in range(N_WARM0):
                nc.tensor.matmul(pw[:], wq_sb[:, 0, 0:128], wq_sb[:, 0, 0:256],
                                 start=(dd == 0), stop=(dd == N_WARM0 - 1))

        # ---- persistent activation tiles ----
        xn_all = persist.tile([128, 16, D], bf16, tag="bigA")   # LN1(x), t-major
        xnT = persist.tile([128, 8, T], bf16, tag="bigB")       # LN1(x)^T
        qT = mid.tile([128, 2, T], bf16, tag="mid")             # q^T (4 heads)
        kT = mid.tile([128, 2, T], bf16, tag="mid")             # k^T
        v_sb = mid.tile([128, HL, 16, 80], fp8, tag="mid")  # v + ones col (80: stride%16)
        oaT = mid.tile([128, 2, T], fp8, tag="mid")             # attn out^T (fp8 wire)

        nc.vector.tensor_copy(
            out=v_sb[:, :, :, DH:DH + 1],
            in_=nc.const_aps.tensor(1.0, (128, HL, 16, 1), f32))

        # per-window AllGather staging (group of 4 cores): one gather per
        # window (both head pairs together).  Distinct tags: staging buffers
        # must never share a pool slot, or a later gather's input DMA
        # serializes on an earlier collective finishing.
        ag_in = [dram.tile([128, 2, TG], fp8, name=f"ag_in{w}",
                           tag=f"ag_in{w}") for w in range(4)]
        ag_out = [dram.tile([4, 128, 2, TG], fp8, name=f"ag_out{w}",
                            tag=f"ag_out{w}") for w in range(4)]

        w1pre = []  # prefetched FFN1 weight tiles
        xrts = []   # prefetched residual-row tiles
        xgpool = top.enter_context(tc.tile_pool(name="xgpre", bufs=4))
        oap = top.enter_context(tc.tile_pool(name="oap", bufs=1))
        oa_bf = oap.tile([128, 8, TG], bf16, tag="oabf")   # blended heads
        oa_w3 = oap.tile([128, 8, TG], fp8, tag="oaw3")    # window-3 gather
        oa_tmp = oap.tile([128, 8, TG], fp8, tag="oatmp")  # gather scratch

        # ========== merged LN1 + QKV + attention pipeline ==========
        with ExitStack() as ph:
            lnw = ph.enter_context(tc.tile_pool(name="lnw", bufs=4))
            lns = ph.enter_context(tc.tile_pool(name="lns", bufs=4))
            epool = ph.enter_context(tc.tile_pool(name="ep", bufs=12))
            spool = ph.enter_context(tc.tile_pool(name="sp", bufs=2))
            ps_q = ph.enter_context(tc.tile_pool(name="psq", bufs=1, space="PSUM"))
            ps_s = ph.enter_context(tc.tile_pool(name="pss", bufs=2, space="PSUM"))
            ps_o = ph.enter_context(tc.tile_pool(name="pso", bufs=2, space="PSUM"))

            def ln_group(g0):
                """LN for 4 consecutive tiles: all DMAs first, one shared
                rsqrt chain, transposes last (keeps queues from ping-ponging)."""
                xts = []
                for j in range(4):
                    to = g0 + j
                    xt = lnw.tile([128, D], bf16, tag="xt", name=f"xt{to}")
                    nc.sync.dma_start(xt[:], x_in[to * 128:(to + 1) * 128, :])
                    xts.append(xt)
                agg4 = lns.tile([128, 4, 2], f32, tag="agg4", name=f"agg4_{g0}")
                for j in range(4):
                    bns = lns.tile([128, 2, 6], f32, tag="bns")
                    for i in range(2):
                        nc.vector.bn_stats(bns[:, i, :],
                                           xts[j][:, i * 512:(i + 1) * 512])
                    nc.vector.bn_aggr(agg4[:, j, :],
                                      bns[:].rearrange("p a b -> p (a b)"))
                ve4 = lns.tile([128, 4], f32, tag="ve4", name=f"ve4_{g0}")
                nc.vector.tensor_scalar(ve4[:], agg4[:, :, 1], EPS, None,
                                        ALU.add)
                rstd4 = _rsqrtN(nc, lns, ve4, magict[:], 4)
                nmr4 = lns.tile([128, 4], f32, tag="nmr4", name=f"nmr4_{g0}")
                nc.vector.tensor_tensor(nmr4[:], agg4[:, :, 0], rstd4[:],
                                        ALU.mult)
                nc.vector.tensor_scalar(nmr4[:], nmr4[:], -1.0, None, ALU.mult)
                for j in range(4):
                    to = g0 + j
                    nc.vector.tensor_scalar(xn_all[:, to, :], xts[j][:],
                                            rstd4[:, j:j + 1], nmr4[:, j:j + 1],
                                            ALU.mult, ALU.add)
                for j in range(4):
                    to = g0 + j
                    nc.sync.dma_start_transpose(
                        xnT[:, :, to * 128:(to + 1) * 128], xn_all[:, to, :])

            def qkv_window_ops(w):
                """Generator: emits one QKV matmul per next()."""
                for dst, w_sb, bias in ((qT, wq_sb, qbt), (kT, wk_sb, kbt)):
                    for mo in range(2):
                        pq = ps_q.tile([128, 512], f32, tag="pq",
                                       name=f"pq{w}_{dst.tensor.name[:2]}_{mo}")
                        for ko in range(8):
                            nc.tensor.matmul(
                                pq[:],
                                w_sb[:, ko, mo * 128:(mo + 1) * 128],
                                xnT[:, ko, w * 512:(w + 1) * 512],
                                start=(ko == 0), stop=(ko == 7))
                            yield
                        nc.vector.tensor_scalar(
                            dst[:, mo, w * 512:(w + 1) * 512], pq[:],
                            bias[:, mo:mo + 1], None, ALU.add)
                for to in range(4 * w, 4 * w + 4):
                    pv = ps_q.tile([128, 512], f32, tag="pq",
                                   name=f"pv{w}_{to}")
                    for ko in range(8):
                        nc.tensor.matmul(pv[:, 0:256],
                                         xnT[:, ko, to * 128:(to + 1) * 128],
                                         wv_sb[:, ko, :],
                                         start=(ko == 0), stop=(ko == 7))
                        yield
                    nc.vector.tensor_tensor(
                        v_sb[:, :, to, 0:DH],
                        pv[:, 0:256].rearrange("p (h d) -> p h d", h=HL),
                        vbb[:], ALU.add)

            ln_group(0)
            for _ in qkv_window_ops(0):
                pass
            ln_group(4)

            warm = ps_q.tile([128, 256], f32, tag="warm", bufs=1,
                             name="warm_acc")

            for w in range(4):
                n_s = 4 * w + 4
                G = n_s // 2
                gen = qkv_window_ops(w + 1) if w < 3 else iter(())
                npts = 3 * n_s
                nops = 48
                quota, rem = divmod(nops, npts) if w < 3 else (1, 0)

                def pull(i):
                    n = quota + (1 if i < rem else 0)
                    for _ in range(n):
                        if next(gen, None) is None:
                            # HAM filler: attention's matmul stream has
                            # sub-us holes (exp/mask deps, LDWEIGHTS), and a
                            # single fully-idle 3.4us window re-throttles the
                            # PE to half clock permanently.  Trickle dummy
                            # matmuls to keep the activity window busy.
                            nc.tensor.matmul(warm[:], wq_sb[:, 0, 0:128],
                                             wq_sb[:, 0, 0:256],
                                             start=True, stop=True)
                            break

                def score_pair(ch, gi):
                    """Scores for head pair (2ch, 2ch+1): the two 64-row
                    matmuls alternate row halves of the PE array, so each
                    adjacent pair of instructions runs concurrently
                    (row tiling)."""
                    pss = [ps_s.tile([128, 2, 512], f32, tag="pss",
                                     name=f"pss{w}_{ch}_{gi}_{hf}")
                           for hf in (0, 1)]
                    for j in (0, 1):
                        it = 2 * gi + j
                        for hf in (0, 1):
                            nc.tensor.matmul(
                                pss[hf][:, j, :],
                                kT[hf * 64:hf * 64 + 64, ch,
                                   it * 128:(it + 1) * 128],
                                qT[hf * 64:hf * 64 + 64, ch,
                                   w * 512:(w + 1) * 512],
                                start=True, stop=True)
                    out = []
                    for hf in (0, 1):
                        et = epool.tile([128, 2, 512], fp8, tag="et",
                                        name=f"et{w}_{ch}_{gi}_{hf}")
                        # exp(s/8)/32: the /32 (exp bias -ln32) keeps outlier
                        # scores inside fp8e4's +-240 range; numerator and
                        # denominator scale together so softmax is unchanged
                        nc.scalar.activation(et[:], pss[hf][:], AF.Exp,
                                             scale=0.125, bias=expb[:])
                        for j in (0, 1):
                            it = 2 * gi + j
                            if it >= 4 * w:
                                # causal: zero where key > query
                                d = it - 4 * w
                                nc.vector.tensor_tensor(
                                    et[:, j, :], et[:, j, :],
                                    cmask[:, d, :], ALU.mult)
                        out.append(et)
                    return out

                pt = 0
                ets = {h: [] for h in range(HL)}
                for gi in range(G):
                    e0, e1 = score_pair(0, gi)
                    ets[0].append(e0)
                    ets[1].append(e1)
                    pull(pt); pt += 1
                for h in range(HL):
                    po, ch = (h % 2) * 64, h // 2
                    pvo = ps_o.tile([128, 512], f32, tag="pvo",
                                    name=f"pvo{h}_{w}")
                    for it in range(n_s):
                        nc.tensor.matmul(
                            pvo[0:DH + 1, :],
                            v_sb[:, h, it, 0:DH + 1],
                            ets[h][it // 2][:, it % 2, :],
                            start=(it == 0), stop=(it == n_s - 1))
                        if h < 2 and it % 2 == 1:
                            k2 = h * n_s + it
                            if k2 % 4 == 1:
                                e0, e1 = score_pair(1, k2 // 4)
                                ets[2].append(e0)
                                ets[3].append(e1)
                                pull(pt); pt += 1
                        pull(pt); pt += 1
                    # softmax normalize: 1/denominator broadcast to 64
                    # partitions via a tiny PE ones-matmul into the upper
                    # half of pvo's PSUM bank (gpsimd stays CC-only)
                    lrow = spool.tile([1, 512], f32, tag="lrow")
                    nc.vector.tensor_copy(lrow[:], pvo[DH:DH + 1, :])
                    lb = spool.tile([64, 512], f32, tag="lb")
                    nc.gpsimd.partition_broadcast(lb[:], lrow[:])
                    nc.vector.reciprocal_approx_fast(out=lb[:], in_=lb[:])
                    nc.vector.tensor_tensor(
                        oaT[po:po + 64, ch, w * 512:(w + 1) * 512],
                        pvo[0:DH, :], lb[:], ALU.mult)
                    if h == HL - 1:
                        # ship this window's 4 heads to their owner rank.
                        # Staging goes through gpsimd's SOFTWARE DGE queues:
                        # a hardware ring that delivered a CC input is not
                        # reusable until that collective completes, and those
                        # waits get hoisted in front of unrelated pipeline
                        # DMAs.  The SW queues carry only CC-adjacent traffic
                        # so the poison can never reach the pipeline.
                        # staging unpinned (sits naturally after this
                        # window's broadcasts in the gpsimd stream); the CC
                        # trigger stays pinned late in the model so its
                        # observation waits land on post-attention guards
                        nc.gpsimd.dma_start(
                            ag_in[w][:],
                            oaT[:, :, w * 512:(w + 1) * 512])
                        with tc.tile_wait_until(0.118 + 0.005 * w):
                            nc.gpsimd.collective_compute(
                                "AllGather", ALU.bypass,
                                replica_groups=GROUPS,
                                ins=[ag_in[w][:].opt()],
                                outs=[ag_out[w][:].opt()])
                if w < 3:
                    for _ in gen:
                        pass
                if w == 0:
                    ln_group(8)
                elif w == 1:
                    ln_group(12)
                    nc.sync.dma_start(wp_sb[:], wp_in[:])
                    for t2 in range(4):
                        xrt = xgpool.tile([128, D], bf16, tag="xrt",
                                          name=f"xrt{t2}")
                        nc.sync.dma_start(xrt[:],
                                          xg_in[t2 * 128:(t2 + 1) * 128, :])
                        xrts.append(xrt)
                elif w == 2:
                    # prefetch the first FFN1 weight tiles
                    for mo in range(6):
                        w1t = wpool.tile([128, 8, 128], bf16, tag="w1t",
                                         name=f"w1t{mo}")
                        nc.sync.dma_start(w1t[:], w1_in[:, mo])
                        w1pre.append(w1t)

            def oa_stitch(dst, wv_):
                """DMA window wv_'s gather into a [128, (rank, pair), t]
                SBUF tile.  Runs on gpsimd's software DGE: it is the ONLY
                engine allowed to carry Collectives-semaphore waits, so a
                slow gather can never stall the compute pipeline."""
                v4 = dst[:].rearrange("p (r c) t -> p r c t", c=2)
                nc.scalar.dma_start(
                    v4[:], ag_out[wv_][:].rearrange("r p c t -> p r c t"))

            # blend windows 0-2 into oa_bf while the last gathers finish;
            # one-hot wsel selects this rank's window (SPMD-safe).
            # tile_wait_until pins the stitches + blends late in the
            # scheduler's modeled timeline: their Collectives waits (and
            # split pre-waits) must never precede pipeline work in any
            # engine stream, or a slow gather stalls the whole attention
            # pipeline (the scheduler's CC cost model is optimistic).
            with tc.tile_wait_until(1.0):
                for wv_ in range(3):
                    oa_stitch(oa_tmp, wv_)
                    if wv_ == 0:
                        nc.vector.tensor_scalar_mul(oa_bf[:], oa_tmp[:],
                                                    wselt[:, 0:1])
                    else:
                        nc.vector.scalar_tensor_tensor(
                            oa_bf[:], oa_tmp[:], wselt[:, wv_:wv_ + 1],
                            oa_bf[:], ALU.mult, ALU.add)

                # PE warm chain bridging the final AllGather
                pbr = ps_q.tile([128, 512], f32, tag="pq", name="bridge_acc")
                for dd in range(N_BRIDGE):
                    nc.tensor.matmul(pbr[:], wq_sb[:, 0, 0:128],
                                     wq_sb[:, 0:2, :],
                                     start=(dd == 0),
                                     stop=(dd == N_BRIDGE - 1))
                oa_stitch(oa_w3, 3)

        # ===== projection of gathered heads + residual + LN2 + transpose =====
        x2 = persist.tile([128, 4, D], f32, tag="bigB")  # x1 rows (FFN residual)
        xn2T = mid.tile([128, 8, TG], bf16, tag="mid")
        with ExitStack() as ph:
            work = ph.enter_context(tc.tile_pool(name="p6work", bufs=2))
            small = ph.enter_context(tc.tile_pool(name="p6small", bufs=4))
            psum = ph.enter_context(tc.tile_pool(name="p6psum", bufs=4,
                                                 space="PSUM"))
            xn2_all = work.tile([128, 4, D], bf16, tag="xn2a", bufs=1)
            for t2 in range(4):
                sl = slice(t2 * 128, (t2 + 1) * 128)
                nc.vector.scalar_tensor_tensor(
                    oa_bf[:, :, sl], oa_w3[:, :, sl], wselt[:, 3:4],
                    oa_bf[:, :, sl], ALU.mult, ALU.add)
                xrt = xrts[t2]
                for no in range(2):
                    pp = psum.tile([128, 512], f32, tag="pp")
                    for ko in range(8):
                        nc.tensor.matmul(
                            pp[:],
                            oa_bf[:, ko, sl],
                            wp_sb[:, ko, no * 512:(no + 1) * 512],
                            start=(ko == 0), stop=(ko == 7))
                    nc.vector.tensor_tensor(
                        x2[:, t2, no * 512:(no + 1) * 512], pp[:],
                        xrt[:, no * 512:(no + 1) * 512], ALU.add)
                rstd, nmr = _ln_stats(nc, small, x2[:, t2, :], D, magict[:])
                nc.scalar.activation(xn2_all[:, t2, :], x2[:, t2, :],
                                     AF.Identity, bias=nmr[:], scale=rstd[:])
                # on the scalar-hosted DGE + pinned late: SP must carry ZERO
                # CC-downstream DMAs, or the scheduler plants incremental
                # Collectives pre-waits in front of the attention pipeline's
                # SP stream and the cold-start barrier stalls everything
                with tc.tile_wait_until(1.0):
                    nc.scalar.dma_start_transpose(
                        xn2T[:, :, t2 * 128:(t2 + 1) * 128], xn2_all[:, t2, :])
            # short warm chain over the LN2 tail before FFN1 starts
            pb2 = psum.tile([128, 512], f32, tag="pp", name="p6bridge")
            for dd in range(16):
                nc.tensor.matmul(pb2[:], wq_sb[:, 0, 0:128], wq_sb[:, 0:2, :],
                                 start=(dd == 0), stop=(dd == 15))

        # ================= FFN first matmul =================
        hT = persist.tile([128, 32, TG], bf16, tag="bigA")
        with ExitStack() as ph:
            psum = ph.enter_context(tc.tile_pool(name="p7psum", bufs=6,
                                                 space="PSUM"))
            for mo in range(32):
                if mo < len(w1pre):
                    w1t = w1pre[mo]
                else:
                    w1t = wpool.tile([128, 8, 128], bf16, tag="w1t",
                                     name=f"w1t{mo}")
                    nc.sync.dma_start(w1t[:], w1_in[:, mo])
                ph_ = psum.tile([128, 512], f32, tag="ph")
                for ko in range(8):
                    nc.tensor.matmul(ph_[:], w1t[:, ko, :], xn2T[:, ko, :],
                                     start=(ko == 0), stop=(ko == 7))
                nc.scalar.activation(hT[:, mo, :], ph_[:], AF.Relu,
                                     bias=b1s[:, mo:mo + 1])

        # ============ FFN second matmul + epilogue ============
        with ExitStack() as ph:
            w2pool = ph.enter_context(tc.tile_pool(name="p8w", bufs=8))
            work = ph.enter_context(tc.tile_pool(name="p8work", bufs=2))
            psum = ph.enter_context(tc.tile_pool(name="p8psum", bufs=8,
                                                 space="PSUM"))
            py = [psum.tile([128, 512], f32, tag="py", name=f"py{i}")
                  for i in range(8)]
            for ko in range(32):
                w2t = w2pool.tile([128, D], bf16, tag="w2t")
                nc.sync.dma_start(w2t[:], w2_in[ko * 128:(ko + 1) * 128, :])
                for m2 in range(4):
                    for no in range(2):
                        nc.tensor.matmul(
                            py[m2 * 2 + no][:],
                            hT[:, ko, m2 * 128:(m2 + 1) * 128],
                            w2t[:, no * 512:(no + 1) * 512],
                            start=(ko == 0), stop=(ko == 31))
            for m2 in range(4):
                osb = work.tile([128, D], f32, tag="osb")
                for no in range(2):
                    sl = slice(no * 512, (no + 1) * 512)
                    nc.vector.tensor_tensor(
                        osb[:, sl], py[m2 * 2 + no][:], x2[:, m2, sl], ALU.add)
                    nc.vector.tensor_tensor(
                        osb[:, sl], osb[:, sl], b2b[:, sl], ALU.add)
                with tc.tile_wait_until(1.0):
                    nc.scalar.dma_start(out_dram[m2 * 128:(m2 + 1) * 128, :],
                                        osb[:])

    nc.compile()
    return nc


def _prep(inputs):
    x = np.asarray(inputs["x"], np.float32)
    wq = np.asarray(inputs["wq"], np.float32)
    wk = np.asarray(inputs["wk"], np.float32)
    wv = np.asarray(inputs["wv"], np.float32)
    wp = np.asarray(inputs["w_proj"], np.float32)
    bp = np.asarray(inputs["b_proj"], np.float32)
    w1 = np.asarray(inputs["w1"], np.float32)
    b1 = np.asarray(inputs["b1"], np.float32)
    w2 = np.asarray(inputs["w2"], np.float32)
    b2 = np.asarray(inputs["b2"], np.float32)
    ln1_g = np.asarray(inputs["ln1_g"], np.float32)
    ln1_b = np.asarray(inputs["ln1_b"], np.float32)
    ln2_g = np.asarray(inputs["ln2_g"], np.float32)
    ln2_b = np.asarray(inputs["ln2_b"], np.float32)

    # fold LN gains into the adjacent weights (host-side)
    w1f = ln2_g[:, None] * w1                     # [1024, 4096]
    b1f = b1 + ln2_b @ w1                         # [4096]
    w1r = np.ascontiguousarray(
        w1f.reshape(8, 128, 32, 128).transpose(1, 2, 0, 3)).astype(BF16)
    wpr = np.ascontiguousarray(
        wp.reshape(8, 128, D).transpose(1, 0, 2)).astype(BF16)
    w2r = w2.astype(BF16)
    b1r = np.ascontiguousarray(b1f.reshape(32, 128).T)

    in_maps = []
    for c in range(N_CORES):
        b, g = divmod(c, 4)
        h0 = HL * g
        wqc = np.concatenate([wq[h] for h in range(h0, h0 + HL)], axis=1)
        wkc = np.concatenate([wk[h] for h in range(h0, h0 + HL)], axis=1)
        wvc = np.concatenate([wv[h] for h in range(h0, h0 + HL)], axis=1)
        qb = ln1_b @ wqc                          # [256]
        kb = ln1_b @ wkc
        vb = ln1_b @ wvc
        wqf = ln1_g[:, None] * wqc
        wkf = ln1_g[:, None] * wkc
        wvf = ln1_g[:, None] * wvc
        wsel = np.zeros((128, 4), np.float32)
        wsel[:, g] = 1.0

        in_maps.append({
            "x": np.ascontiguousarray(x[b]).astype(BF16),
            "xg": np.ascontiguousarray(x[b, g * TG:(g + 1) * TG, :]
                                       + bp[None, :]).astype(BF16),
            "wq": np.ascontiguousarray(
                wqf.reshape(8, 128, HL * DH).transpose(1, 0, 2)).astype(BF16),
            "wk": np.ascontiguousarray(
                wkf.reshape(8, 128, HL * DH).transpose(1, 0, 2)).astype(BF16),
            "wv": np.ascontiguousarray(
                wvf.reshape(8, 128, HL * DH).transpose(1, 0, 2)).astype(BF16),
            "qb": np.ascontiguousarray(qb.reshape(2, 128).T),
            "kb": np.ascontiguousarray(kb.reshape(2, 128).T),
            "vb": np.ascontiguousarray(vb.reshape(1, HL * DH)),
            "wp": wpr,
            "wsel": wsel,
            "w1": w1r,
            "b1": b1r,
            "w2": w2r,
            "b2": np.ascontiguousarray(b2.reshape(1, D)),
        })
    return in_maps


def _make_runner(nc):
    """Build a cached jitted SPMD executor (mirrors bass2jax.run_bass_via_pjrt
    but jits once and is reused across kernel() calls)."""
    import jax
    from jax.experimental.shard_map import shard_map
    from jax.sharding import Mesh, PartitionSpec
    from concourse import bass2jax as b2j

    b2j.install_neuronx_cc_hook()
    partition_name = (nc.partition_id_tensor.name
                      if nc.partition_id_tensor else None)
    in_names, out_names, out_avals, zero_shapes = [], [], [], []
    for alloc in nc.m.functions[0].allocations:
        if not isinstance(alloc, mybir.MemoryLocationSet):
            continue
        name = alloc.memorylocations[0].name
        if alloc.kind == "ExternalInput":
            if name != partition_name:
                in_names.append(name)
        elif alloc.kind == "ExternalOutput":
            shape = tuple(alloc.tensor_shape)
            dtype = mybir.dt.np(alloc.dtype)
            out_names.append(name)
            out_avals.append(jax.core.ShapedArray(shape, dtype))
            zero_shapes.append((shape, dtype))
    n_params = len(in_names)
    n_outs = len(out_avals)
    all_in_names = list(in_names) + list(out_names)
    if partition_name is not None:
        all_in_names.append(partition_name)
    donate = tuple(range(n_params, n_params + n_outs))

    def _body(*args):
        operands = list(args)
        if partition_name is not None:
            operands.append(b2j.partition_id_tensor())
        outs = b2j._bass_exec_p.bind(
            *operands,
            out_avals=tuple(out_avals),
            in_names=tuple(all_in_names),
            out_names=tuple(out_names),
            lowering_input_output_aliases=(),
            sim_require_finite=True,
            sim_require_nnan=True,
            nc=nc,
        )
        return tuple(outs)

    devices = jax.devices()[:N_CORES]
    mesh = Mesh(np.asarray(devices), ("core",))
    in_specs = (PartitionSpec("core"),) * (n_params + n_outs)
    out_specs = (PartitionSpec("core"),) * n_outs
    sharded = jax.jit(
        shard_map(_body, mesh=mesh, in_specs=in_specs, out_specs=out_specs,
                  check_rep=False),
        donate_argnums=donate, keep_unused=True)

    def run(in_maps):
        concat_in = [
            np.concatenate([np.asarray(in_maps[c][name])
                            for c in range(N_CORES)], axis=0)
            for name in in_names
        ]
        concat_zeros = [
            np.zeros((N_CORES * s[0], *s[1:]), dt) for s, dt in zero_shapes
        ]
        out_arrs = sharded(*concat_in, *concat_zeros)
        return [
            {name: np.asarray(out_arrs[i]).reshape(N_CORES,
                                                   *zero_shapes[i][0])[c]
             for i, name in enumerate(out_names)}
            for c in range(N_CORES)
        ]

    return run


_CACHE = {}


def _get_nc():
    if "nc" not in _CACHE:
        _CACHE["nc"] = _build()
    return _CACHE["nc"]


def _get_runner():
    if "run" not in _CACHE:
        _CACHE["run"] = _make_runner(_get_nc())
    return _CACHE["run"]


def kernel(**inputs):
    run = _get_runner()
    in_maps = _prep(inputs)
    res = run(in_maps)
    B = 2
    out = np.empty((B, T, D), np.float32)
    for c in range(N_CORES):
        b, g = divmod(c, 4)
        out[b, g * TG:(g + 1) * TG, :] = res[c]["out"]
    return out

